# revision 1
# baseline (speedup 1.0000x reference)
"""Trainium2 Bass kernel for nn_AttEncoderLayer (B=8,L=1024,D=256,H=8,DK=DV=32,DF=1024).

Sharding: data-parallel over batch, 1 batch element per NeuronCore (8 cores).
Layout strategy: everything transposed ([feature, token]) on device so that
softmax runs in S^T = [keys, queries] layout:
  - padding mask folds into the ACT exp per-partition bias (keys = partitions)
  - softmax denominators come free from an appended ones-column in V (M=33
    col-tiled attn@V matmuls)
  - LN stats via ones-matmuls (contraction over the feature=partition dim)
Matmuls run as float32r (full PE rate); everything else fp32.
"""

import sys, os
import numpy as np

if "/opt/trn_rl_repo" not in sys.path:
    sys.path.insert(0, "/opt/trn_rl_repo")

from contextlib import ExitStack

import concourse.bass as bass
import concourse.bacc as bacc
import concourse.tile as tile
from concourse import mybir
from concourse.bass_utils import run_bass_kernel_spmd
from concourse.masks import make_identity

f32 = mybir.dt.float32
f32r = mybir.dt.float32r
bf16 = mybir.dt.bfloat16
AF = mybir.ActivationFunctionType
ALU = mybir.AluOpType

B, L, D, H, DK, DV, DF = 8, 1024, 256, 8, 32, 32, 1024
N_CORES = 8
NKT = 8          # key tiles of 128
SCALE = 1.0 / 16.0   # 1/sqrt(d_model)
MASK_BIAS = -30000.0
EPS = 1e-3

_PROGRAM_CACHE = {}


def _r(ap):
    return ap.bitcast(f32r)


def build_program(mask_mode, skips, ln1_trivial, ln2_trivial, debug=False):
    """mask_mode: 'bias' (q-constant padding mask) or 'full' (general mask).
    skips: tuple of fully-masked k-tile indices (skipped entirely)."""
    nc = bacc.Bacc("TRN2", target_bir_lowering=False, debug=False,
                   num_devices=N_CORES)

    xT_d = nc.dram_tensor("xT", [D, L], f32, kind="ExternalInput").ap()
    wq_d = nc.dram_tensor("wq", [128, 512], f32, kind="ExternalInput").ap()
    wk_d = nc.dram_tensor("wk", [128, 512], f32, kind="ExternalInput").ap()
    wv_d = nc.dram_tensor("wv", [128, 512], f32, kind="ExternalInput").ap()
    pw_d = nc.dram_tensor("pw", [128, 1024], f32, kind="ExternalInput").ap()
    w1_d = nc.dram_tensor("w1t", [128, 2048], f32, kind="ExternalInput").ap()
    w2_d = nc.dram_tensor("w2t", [128, 2048], f32, kind="ExternalInput").ap()
    b1_d = nc.dram_tensor("b1v", [128, 8], f32, kind="ExternalInput").ap()
    v2_d = nc.dram_tensor("vec2", [128, 8], f32, kind="ExternalInput").ap()
    cst_d = nc.dram_tensor("cst", [128, 320], f32, kind="ExternalInput").ap()
    if mask_mode == "bias":
        mb_d = nc.dram_tensor("mb", [128, 8], f32, kind="ExternalInput").ap()
    else:
        mb_d = nc.dram_tensor("mbT", [L, L], f32, kind="ExternalInput").ap()
    if not ln2_trivial:
        ln2_d = nc.dram_tensor("ln2v", [2, D], f32, kind="ExternalInput").ap()
    out_d = nc.dram_tensor("out", [L, D], f32, kind="ExternalOutput").ap()
    if debug:
        dbg = {n: nc.dram_tensor(f"dbg_{n}", shp, f32, kind="ExternalOutput").ap()
               for n, shp in [("qT", [128, 2048]), ("kT", [128, 2048]),
                              ("v", [128, 1056]), ("oT", [128, 4096]),
                              ("y1T", [128, 2048]), ("o1T", [128, 2048]),
                              ("hmT", [128, 8192]), ("z2T", [128, 2048]),
                              ("rowflat", [1, 2048]), ("w1t", [128, 48]),
                              ("w2t", [128, 48]), ("pt00", [128, 512]),
                              ("den00", [1, 2048])]}

    kts = [k for k in range(NKT) if k not in skips]

    with tile.TileContext(nc) as tc:
        with ExitStack() as ctx:
            sb = ctx.enter_context(tc.tile_pool(name="sb", bufs=1))
            ps = ctx.enter_context(tc.tile_pool(name="ps", bufs=1, space="PSUM"))

            # ---------- persistent SBUF tensors ----------
            ident = sb.tile([128, 128], f32, tag="ident")
            make_identity(nc, ident)
            ones_col = sb.tile([128, 1], f32, tag="ones_col")
            nc.sync.dma_start(_r(ones_col[:]), _r(cst_d[:, 0:1]))
            zbf = sb.tile([128, 512], bf16, tag="zbf")
            nc.sync.dma_start(zbf[:], cst_d[:, 64:320].bitcast(bf16))
            scratch = sb.tile([128, 8], f32, tag="scratch")
            nc.gpsimd.memset(scratch[:], 0.0)
            # dummy exp: forces the exp table load early, overlapping DMAs
            nc.scalar.activation(scratch[:, 0:1], scratch[:, 0:1], AF.Exp)

            xT_s = sb.tile([128, 2048], f32, tag="xT")   # chunk c at 1024c
            nc.sync.dma_start(_r(xT_s[:, 0:1024]), _r(xT_d[0:128, :]))
            nc.sync.dma_start(_r(xT_s[:, 1024:2048]), _r(xT_d[128:256, :]))
            wq_s = sb.tile([128, 512], f32, tag="wq")
            nc.sync.dma_start(_r(wq_s[:]), _r(wq_d[:]))
            wk_s = sb.tile([128, 512], f32, tag="wk")
            nc.sync.dma_start(_r(wk_s[:]), _r(wk_d[:]))
            wv_s = sb.tile([128, 512], f32, tag="wv")
            nc.sync.dma_start(_r(wv_s[:]), _r(wv_d[:]))
            pw_s = sb.tile([128, 1024], f32, tag="pw")
            nc.sync.dma_start(_r(pw_s[:]), _r(pw_d[:]))
            w1_s = sb.tile([128, 2048], f32, tag="w1t")
            nc.sync.dma_start(_r(w1_s[:]), _r(w1_d[:]))
            w2_s = sb.tile([128, 2048], f32, tag="w2t")
            nc.sync.dma_start(_r(w2_s[:]), _r(w2_d[:]))
            b1_s = sb.tile([128, 8], f32, tag="b1v")
            nc.sync.dma_start(b1_s[:], b1_d[:])
            v2_s = sb.tile([128, 8], f32, tag="vec2")
            nc.sync.dma_start(v2_s[:], v2_d[:])
            if mask_mode == "bias":
                mb_s = sb.tile([128, 8], f32, tag="mb")
                nc.sync.dma_start(mb_s[:], mb_d[:])
            else:
                mb_s = sb.tile([128, 8192], f32, tag="mbT")  # kt block at 1024*kt
                for kt in kts:
                    nc.sync.dma_start(mb_s[:, 1024 * kt:1024 * (kt + 1)],
                                      mb_d[128 * kt:128 * (kt + 1), :])
            if not ln2_trivial:
                ln2_s = sb.tile([1, 2 * D], f32, tag="ln2v")
                nc.sync.dma_start(ln2_s[0:1, :],
                                  ln2_d[:].rearrange("a x -> (a x)").unsqueeze(0))
                a2rep = sb.tile([128, D], f32, tag="a2rep")
                nc.gpsimd.partition_broadcast(a2rep[:], ln2_s[0:1, 0:D])
                b2rep = sb.tile([128, D], f32, tag="b2rep")
                nc.gpsimd.partition_broadcast(b2rep[:], ln2_s[0:1, D:2 * D])

            qT_s = sb.tile([128, 2048], f32, tag="qT")   # hg block at 1024hg
            kT_s = sb.tile([128, 2048], f32, tag="kT")
            # V (bf16): [k-tile 128, 8 heads x 33 (32 v cols + ones col)]
            v_s = sb.tile([128, 8 * 264], bf16, tag="v")
            vv = v_s.rearrange("p (a x) -> p a x", x=33)
            # ones columns MUST be written by the same engine as the V copies:
            # a DMA writing 2-byte elements that share 32-bit words with
            # DVE-written bytes corrupts isolated elements (sub-word
            # concurrent-write hazard, observed on HW)
            nc.vector.memset(vv[:, :, 32:33], 1.0)

            oT_s = sb.tile([128, 4096], f32, tag="oT")   # block b=2hg+t at 1024b
            y1T_s = sb.tile([128, 2048], f32, tag="y1T")
            o1T_s = sb.tile([128, 2048], f32, tag="o1T")
            hmT_s = sb.tile([128, 8192], f32, tag="hmT")  # df tile at 1024*tf
            z2T_s = sb.tile([128, 2048], f32, tag="z2T")

            # ---------- phase 1: Q^T, K^T, V ----------
            for hg in range(2):
                qp = ps.tile([128, 1024], f32, tag="mm", bufs=2, name=f"qp{hg}")
                for n in range(2):
                    for c in range(2):
                        nc.tensor.matmul(
                            qp[:, 512 * n:512 * (n + 1)],
                            _r(wq_s[:, 128 * (2 * hg + c):128 * (2 * hg + c + 1)]),
                            _r(xT_s[:, 1024 * c + 512 * n:1024 * c + 512 * (n + 1)]),
                            start=(c == 0), stop=(c == 1))
                nc.vector.tensor_copy(_r(qT_s[:, 1024 * hg:1024 * (hg + 1)]), _r(qp[:]))
                kp = ps.tile([128, 1024], f32, tag="mm", bufs=2, name=f"kp{hg}")
                for n in range(2):
                    for c in range(2):
                        nc.tensor.matmul(
                            kp[:, 512 * n:512 * (n + 1)],
                            _r(wk_s[:, 128 * (2 * hg + c):128 * (2 * hg + c + 1)]),
                            _r(xT_s[:, 1024 * c + 512 * n:1024 * c + 512 * (n + 1)]),
                            start=(c == 0), stop=(c == 1))
                nc.vector.tensor_copy(_r(kT_s[:, 1024 * hg:1024 * (hg + 1)]), _r(kp[:]))
            for kp2 in range(4):   # pairs of k-tiles
                vp = ps.tile([128, 512], f32, tag="st", bufs=2, name=f"vp{kp2}")
                for i in range(2):
                    kt = 2 * kp2 + i
                    for c in range(2):
                        nc.tensor.matmul(
                            vp[:, 256 * i:256 * (i + 1)],
                            _r(xT_s[:, 1024 * c + 128 * kt:1024 * c + 128 * (kt + 1)]),
                            _r(wv_s[:, 256 * c:256 * (c + 1)]),
                            start=(c == 0), stop=(c == 1))
                # scatter into v_s with 33-stride (skip ones cols)
                dst = v_s[:, 264 * 2 * kp2:264 * 2 * (kp2 + 1)]
                dst = dst.rearrange("p (a x) -> p a x", x=33)[:, :, 0:32]
                src = vp.rearrange("p (a x) -> p a x", x=32)
                nc.vector.tensor_copy(dst, src)

            # ---------- phase 2: attention ----------
            # all four (hg, t) denominator pairs collect here (rows 0/32/64/96
            # so they are legal partition-broadcast sources); normalization is
            # deferred past the attention loop so the Ln/Exp activation-table
            # loads happen exactly once each
            denf_all = sb.tile([97, 2048], f32, tag="denf_all")
            nc.gpsimd.memset(denf_all[:], 1.0)
            # dense dummy matmuls bridge the PE-idle gap after QKV so the
            # HAM clock gate stays at K=8/8 (2.4 GHz) through attention
            warm = ps.tile([128, 512], f32, tag="st", bufs=2, name="warm")
            for w in range(16):
                nc.tensor.matmul(warm[:], zbf[:, 0:128], zbf[:],
                                 start=True, stop=True, skip_group_check=True)
            for hg in range(2):
                ov = []
                for t in range(2):
                    ovt = ps.tile([128, 1024], f32, tag="mm", bufs=2,
                                  name=f"ov{hg}{t}")
                    ov.append(ovt)
                    # zero-init full tile so all 128 partitions are defined
                    for n in range(2):
                        nc.tensor.matmul(
                            ovt[:, 512 * n:512 * (n + 1)],
                            zbf[:, 0:128], zbf[:],
                            start=True, stop=False, skip_group_check=True)
                for ki, kt in enumerate(kts):
                    for hp in range(2):   # head pairs: interleave so the two
                        sts, pts = [], []  # heads' matmuls run concurrently
                        for hh in (2 * hp, 2 * hp + 1):
                            st = ps.tile([128, 1024], f32, tag="st", bufs=2,
                                         name=f"st{hg}{kt}{hh}")
                            sts.append(st)
                        for j, hh in enumerate((2 * hp, 2 * hp + 1)):
                            for n in range(2):
                                nc.tensor.matmul(
                                    sts[j][:, 512 * n:512 * (n + 1)],
                                    _r(kT_s[32 * hh:32 * (hh + 1),
                                            1024 * hg + 128 * kt:1024 * hg + 128 * (kt + 1)]),
                                    _r(qT_s[32 * hh:32 * (hh + 1),
                                            1024 * hg + 512 * n:1024 * hg + 512 * (n + 1)]),
                                    start=True, stop=True,
                                    tile_position=(32 * hh, 0))
                        for j, hh in enumerate((2 * hp, 2 * hp + 1)):
                            pt = sb.tile([128, 1024], bf16, tag="pt", bufs=4,
                                         name=f"pt{hg}{kt}{hh}")
                            pts.append(pt)
                            if mask_mode == "bias":
                                nc.scalar.activation(pt[:], sts[j][:], AF.Exp,
                                                     bias=mb_s[:, kt:kt + 1],
                                                     scale=SCALE)
                            else:
                                nc.vector.scalar_tensor_tensor(
                                    sts[j][:], sts[j][:], SCALE,
                                    mb_s[:, 1024 * kt:1024 * (kt + 1)],
                                    ALU.mult, ALU.add)
                                nc.scalar.activation(pt[:], sts[j][:], AF.Exp)
                        for j, hh in enumerate((2 * hp, 2 * hp + 1)):
                            h = 4 * hg + hh
                            ovt, rowbase = ov[hh // 2], 64 * (hh % 2)
                            for n in range(2):
                                nc.tensor.matmul(
                                    ovt[rowbase:rowbase + 33, 512 * n:512 * (n + 1)],
                                    v_s[:, 264 * kt + 33 * h:264 * kt + 33 * (h + 1)],
                                    pts[j][:, 512 * n:512 * (n + 1)],
                                    start=False, stop=(ki == len(kts) - 1),
                                    skip_group_check=True)
                # epilogue: normalize by the denominators (rows 32 and 96)
                for t in range(2):
                    den = sb.tile([128, 1024], f32, tag="den", bufs=2,
                                  name=f"den{hg}{t}")
                    nc.scalar.activation(den[32:33, :], ov[t][32:33, :], AF.Copy)
                    nc.vector.tensor_copy(den[96:97, :], ov[t][96:97, :])
                    r = 32 * (2 * hg + t)
                    nc.sync.dma_start(denf_all[r:r + 1, 0:1024], den[32:33, :])
                    nc.sync.dma_start(denf_all[r:r + 1, 1024:2048],
                                      den[96:97, :])
                    # evict the unnormalized accumulators into staging (the
                    # hmT region is not written until FFN1); the normalize
                    # multiply must NOT be in-place -- in-place DVE ops have
                    # a read/write race on HW
                    blk = 1024 * (2 * hg + t)
                    nc.vector.tensor_copy(_r(hmT_s[:, blk:blk + 1024]),
                                          ov[t][:])
                # per-hg 1/denominator + normalize: hg0's chain hides under
                # hg1's attention. Ln/Exp run on the contiguous 33-partition
                # range covering this hg's two denominator rows.
                base = 64 * hg
                nc.scalar.activation(denf_all[base:base + 33, :],
                                     denf_all[base:base + 33, :], AF.Ln)
                nc.scalar.activation(denf_all[base:base + 33, :],
                                     denf_all[base:base + 33, :], AF.Exp,
                                     scale=-1.0)
                for t in range(2):
                    b_ = 2 * hg + t
                    r = 32 * b_
                    denhop = sb.tile([1, 2048], f32, tag="denhop", bufs=1,
                                     name=f"denhop{b_}")
                    nc.sync.dma_start(denhop[0:1, :], denf_all[r:r + 1, :])
                    repa = sb.tile([128, 1024], f32, tag="repa", bufs=1,
                                   name=f"repa{b_}")
                    nc.gpsimd.partition_broadcast(repa[:], denhop[0:1, 0:1024])
                    repb = sb.tile([128, 1024], f32, tag="repb", bufs=1,
                                   name=f"repb{b_}")
                    nc.gpsimd.partition_broadcast(repb[:], denhop[0:1, 1024:2048])
                    blk = 1024 * b_
                    nc.vector.tensor_mul(_r(oT_s[0:64, blk:blk + 1024]),
                                         hmT_s[0:64, blk:blk + 1024],
                                         repa[0:64, :])
                    nc.vector.tensor_mul(_r(oT_s[64:128, blk:blk + 1024]),
                                         hmT_s[64:128, blk:blk + 1024],
                                         repb[64:128, :])

            # ---------- phase 3: proj + residual + LN1 ----------
            for t in range(2):
                pp = ps.tile([128, 1024], f32, tag="mm", bufs=2, name=f"pp{t}")
                for n in range(2):
                    for c in range(4):
                        nc.tensor.matmul(
                            pp[:, 512 * n:512 * (n + 1)],
                            _r(pw_s[:, 128 * (2 * c + t):128 * (2 * c + t + 1)]),
                            _r(oT_s[:, 1024 * c + 512 * n:1024 * c + 512 * (n + 1)]),
                            start=(c == 0), stop=(c == 3))
                nc.vector.scalar_tensor_tensor(
                    _r(y1T_s[:, 1024 * t:1024 * (t + 1)]), pp[:],
                    v2_s[:, t:t + 1], xT_s[:, 1024 * t:1024 * (t + 1)],
                    ALU.add, ALU.add)

            def ln_stats(src_s, stats_name):
                """src_s: [128, 2048] (two d-chunks). Returns psum [128,16]:
                col 2j = sum over d for q-tile j, col 2j+1 = sum of squares."""
                stp = ps.tile([128, 1024], f32, tag="st", bufs=2,
                              name=f"stats_{stats_name}")
                stq = ps.tile([128, 1024], f32, tag="st", bufs=2,
                              name=f"statq_{stats_name}")
                sqs = []
                for t in range(2):
                    sq = sb.tile([128, 1024], f32, tag="sq", bufs=2,
                                 name=f"sq_{stats_name}{t}")
                    nc.vector.tensor_mul(_r(sq[:]), src_s[:, 1024 * t:1024 * (t + 1)],
                                         src_s[:, 1024 * t:1024 * (t + 1)])
                    sqs.append(sq)
                for n in range(2):
                    for t in range(2):
                        nc.tensor.matmul(
                            stp[0:1, 512 * n:512 * (n + 1)], _r(ones_col[:]),
                            _r(src_s[:, 1024 * t + 512 * n:1024 * t + 512 * (n + 1)]),
                            start=(t == 0), stop=(t == 1), tile_position=(0, 0),
                            skip_group_check=True)
                        nc.tensor.matmul(
                            stq[0:1, 512 * n:512 * (n + 1)], _r(ones_col[:]),
                            _r(sqs[t][:, 512 * n:512 * (n + 1)]),
                            start=(t == 0), stop=(t == 1), tile_position=(0, 0),
                            skip_group_check=True)
                strow = sb.tile([128, 1024], f32, tag="strow", bufs=1,
                                name=f"strow_{stats_name}")
                strowq = sb.tile([128, 1024], f32, tag="strowq", bufs=1,
                                 name=f"strowq_{stats_name}")
                nc.scalar.activation(strowq[0:1, :], stq[0:1, :], AF.Copy)
                nc.vector.tensor_copy(strow[0:1, :], stp[0:1, :])
                stt = ps.tile([128, 1024], f32, tag="st", bufs=2,
                              name=f"stt_{stats_name}")
                for j in range(8):
                    nc.tensor.transpose(stt[:, 2 * j:2 * j + 1],
                                        strow[0:1, 128 * j:128 * (j + 1)],
                                        ident[0:1, 0:1])
                    nc.tensor.transpose(stt[:, 2 * j + 1:2 * j + 2],
                                        strowq[0:1, 128 * j:128 * (j + 1)],
                                        ident[0:1, 0:1])
                return stt

            stt1 = ln_stats(y1T_s, "ln1")
            # per-partition scalar chain on [128, 8]
            w1t = sb.tile([128, 48], f32, tag="lnwork", name="w1t_ln1")
            s_ = stt1[:, 0:16:2]
            q_ = stt1[:, 1:16:2]
            nc.scalar.activation(w1t[:, 0:8], s_, AF.Square)             # sum^2
            nc.vector.scalar_tensor_tensor(w1t[:, 8:16], w1t[:, 0:8],
                                           -1.0 / 256.0, q_, ALU.mult, ALU.add)
            nc.scalar.activation(w1t[:, 16:24], w1t[:, 8:16], AF.Sqrt,
                                 scale=1.0 / 255.0)                      # sigma
            nc.vector.tensor_scalar(w1t[:, 24:32], w1t[:, 16:24], EPS, None,
                                    ALU.add)
            nc.vector.reciprocal(w1t[:, 32:40], w1t[:, 24:32])           # rinv
            nc.vector.scalar_tensor_tensor(w1t[:, 40:48], s_, -1.0 / 256.0,
                                           w1t[:, 32:40], ALU.mult,
                                           ALU.mult)                     # -mu*rinv
            # transpose [128, 8] -> rows [8, 128] for partition broadcast
            rowp = ps.tile([128, 1024], f32, tag="st", bufs=2, name="rowp_ln1")
            nc.tensor.transpose(rowp[0:8, 0:128], w1t[:, 32:40], ident[:])
            nc.tensor.transpose(rowp[0:8, 128:256], w1t[:, 40:48], ident[:])
            rows = sb.tile([8, 256], f32, tag="rows", name="rows_ln1")
            nc.vector.tensor_copy(rows[:], rowp[0:8, 0:256])
            # compute engines can only address SBUF partition bases {0,32,64,96};
            # DMA is exempt -- flatten the 8 rows into partition 0's free dim
            rowflat = sb.tile([1, 2048], f32, tag="rowflat")
            for j in range(8):
                nc.sync.dma_start(rowflat[0:1, 128 * j:128 * (j + 1)],
                                  rows[j:j + 1, 0:128])
                nc.sync.dma_start(rowflat[0:1, 1024 + 128 * j:1024 + 128 * (j + 1)],
                                  rows[j:j + 1, 128:256])
            rrep1 = sb.tile([128, 1024], f32, tag="rrep1")
            mrep1 = sb.tile([128, 1024], f32, tag="mrep1")
            nc.gpsimd.partition_broadcast(rrep1[:], rowflat[0:1, 0:1024])
            nc.gpsimd.partition_broadcast(mrep1[:], rowflat[0:1, 1024:2048])
            for t in range(2):
                blk = slice(1024 * t, 1024 * (t + 1))
                nc.vector.tensor_mul(_r(o1T_s[:, blk]), y1T_s[:, blk],
                                     rrep1[:])
                nc.vector.tensor_add(_r(o1T_s[:, blk]), o1T_s[:, blk],
                                     mrep1[:])
                if not ln1_trivial:
                    nc.vector.tensor_scalar(_r(o1T_s[:, blk]), o1T_s[:, blk],
                                            v2_s[:, 4 + t:5 + t],
                                            v2_s[:, 6 + t:7 + t],
                                            ALU.mult, ALU.add)

            # ---------- phase 4: FFN1 (+bias+relu) ----------
            warm2 = ps.tile([128, 512], f32, tag="st", bufs=2, name="warm2")
            for w in range(8):
                nc.tensor.matmul(warm2[:], zbf[:, 0:128], zbf[:],
                                 start=True, stop=True, skip_group_check=True)
            for tf in range(8):
                hp = ps.tile([128, 1024], f32, tag="mm", bufs=2, name=f"hp{tf}")
                for n in range(2):
                    for c in range(2):
                        nc.tensor.matmul(
                            hp[:, 512 * n:512 * (n + 1)],
                            _r(w1_s[:, 128 * (8 * c + tf):128 * (8 * c + tf + 1)]),
                            _r(o1T_s[:, 1024 * c + 512 * n:1024 * c + 512 * (n + 1)]),
                            start=(c == 0), stop=(c == 1))
                dst = hmT_s[:, 1024 * tf:1024 * (tf + 1)]
                if tf % 2 == 0:
                    nc.vector.tensor_scalar(_r(dst), hp[:], b1_s[:, tf:tf + 1],
                                            0.0, ALU.add, ALU.max)
                else:
                    nc.scalar.activation(_r(dst), hp[:], AF.Relu,
                                         bias=b1_s[:, tf:tf + 1])

            # ---------- phase 5: FFN2 + residual ----------
            for t in range(2):
                op2 = ps.tile([128, 1024], f32, tag="mm", bufs=2, name=f"op2{t}")
                for n in range(2):
                    for c in range(8):
                        nc.tensor.matmul(
                            op2[:, 512 * n:512 * (n + 1)],
                            _r(w2_s[:, 128 * (2 * c + t):128 * (2 * c + t + 1)]),
                            _r(hmT_s[:, 1024 * c + 512 * n:1024 * c + 512 * (n + 1)]),
                            start=(c == 0), stop=(c == 7))
                nc.vector.scalar_tensor_tensor(
                    _r(z2T_s[:, 1024 * t:1024 * (t + 1)]), op2[:],
                    v2_s[:, 2 + t:3 + t], o1T_s[:, 1024 * t:1024 * (t + 1)],
                    ALU.add, ALU.add)

            # ---------- phase 6: LN2 stats ----------
            stt2 = ln_stats(z2T_s, "ln2")
            w2t = sb.tile([128, 48], f32, tag="lnwork2", name="w2t_ln2")
            s2_ = stt2[:, 0:16:2]
            q2_ = stt2[:, 1:16:2]
            nc.scalar.activation(w2t[:, 0:8], s2_, AF.Square)
            nc.vector.scalar_tensor_tensor(w2t[:, 8:16], w2t[:, 0:8],
                                           -1.0 / 256.0, q2_, ALU.mult, ALU.add)
            nc.scalar.activation(w2t[:, 16:24], w2t[:, 8:16], AF.Sqrt,
                                 scale=1.0 / 255.0)
            nc.vector.tensor_scalar(w2t[:, 24:32], w2t[:, 16:24], EPS, None,
                                    ALU.add)
            nc.vector.reciprocal(w2t[:, 32:40], w2t[:, 24:32])           # rinv2
            nc.vector.tensor_scalar(w2t[:, 40:48], s2_, -1.0 / 256.0, None,
                                    ALU.mult)                            # -mu2

            if debug:
                nc.sync.dma_start(dbg["qT"][:], qT_s[:])
                nc.sync.dma_start(dbg["kT"][:], kT_s[:])
                nc.sync.dma_start(dbg["v"][:], v_s[:].bitcast(f32))
                nc.sync.dma_start(dbg["oT"][:], oT_s[:])
                nc.sync.dma_start(dbg["y1T"][:], y1T_s[:])
                nc.sync.dma_start(dbg["o1T"][:], o1T_s[:])
                nc.sync.dma_start(dbg["hmT"][:], hmT_s[:])
                nc.sync.dma_start(dbg["z2T"][:], z2T_s[:])
                nc.sync.dma_start(dbg["rowflat"][:], rowflat[:])
                nc.sync.dma_start(dbg["w1t"][:], w1t[:])
                nc.sync.dma_start(dbg["w2t"][:], w2t[:])

            # ---------- phase 7: transpose back + apply LN2 + store ----------
            for j in range(8):
                zp = ps.tile([128, 256], f32, tag="st", bufs=2, name=f"zp{j}")
                for c in range(2):
                    nc.tensor.transpose(
                        zp[:, 128 * c:128 * (c + 1)],
                        z2T_s[:, 1024 * c + 128 * j:1024 * c + 128 * (j + 1)],
                        ident[:])
                ys = sb.tile([128, 256], f32, tag="ys", bufs=2, name=f"ys{j}")
                nc.vector.tensor_scalar(ys[:], zp[:], w2t[:, 40 + j:41 + j],
                                        w2t[:, 32 + j:33 + j], ALU.add, ALU.mult)
                if not ln2_trivial:
                    nc.vector.tensor_mul(ys[:], ys[:], a2rep[:])
                    nc.vector.tensor_add(ys[:], ys[:], b2rep[:])
                nc.sync.dma_start(out_d[128 * j:128 * (j + 1), :], ys[:])

    nc.compile()
    return nc


def prep_host(inputs):
    """Shared (per-weights) host-side packing. Returns dict of device arrays
    + per-core arrays + build flags."""
    x = np.asarray(inputs["enc_input"], dtype=np.float32)
    mask = np.asarray(inputs["slf_attn_mask"])
    w_qs = np.asarray(inputs["w_qs"], dtype=np.float32)
    w_ks = np.asarray(inputs["w_ks"], dtype=np.float32)
    w_vs = np.asarray(inputs["w_vs"], dtype=np.float32)
    proj_w = np.asarray(inputs["proj_w"], dtype=np.float32)
    proj_b = np.asarray(inputs["proj_b"], dtype=np.float32)
    ln1_a = np.asarray(inputs["ln1_a"], dtype=np.float32)
    ln1_b = np.asarray(inputs["ln1_b"], dtype=np.float32)
    w1 = np.asarray(inputs["w1"], dtype=np.float32)
    b1 = np.asarray(inputs["b1"], dtype=np.float32)
    w2 = np.asarray(inputs["w2"], dtype=np.float32)
    b2 = np.asarray(inputs["b2"], dtype=np.float32)
    ln2_a = np.asarray(inputs["ln2_a"], dtype=np.float32)
    ln2_b = np.asarray(inputs["ln2_b"], dtype=np.float32)

    def pack_qk(w):
        arr = w.transpose(1, 0, 2).reshape(D, H * DK)  # [d, (h dk)]
        outp = np.zeros((128, 512), dtype=np.float32)
        for hg in range(2):
            for c in range(2):
                outp[:, 128 * (2 * hg + c):128 * (2 * hg + c + 1)] = \
                    arr[128 * c:128 * (c + 1), 128 * hg:128 * (hg + 1)]
        return outp

    wq_np = pack_qk(w_qs)
    wk_np = pack_qk(w_ks)
    arr_v = w_vs.transpose(1, 0, 2).reshape(D, H * DV)
    wv_np = np.zeros((128, 512), dtype=np.float32)
    for c in range(2):
        wv_np[:, 256 * c:256 * (c + 1)] = arr_v[128 * c:128 * (c + 1), :]

    # proj lhsT chunks with the oT block layout: block b=2hg+t holds heads
    # (2b, 2b+1) at partitions {0:32, 64:96}; other partitions zero.
    pwT = proj_w.T.astype(np.float32)  # [hv, dm]
    pw_np = np.zeros((128, 1024), dtype=np.float32)
    for b_ in range(4):
        for t in range(2):
            blk = np.zeros((128, 128), dtype=np.float32)
            blk[0:32, :] = pwT[64 * b_:64 * b_ + 32, 128 * t:128 * (t + 1)]
            blk[64:96, :] = pwT[64 * b_ + 32:64 * b_ + 64, 128 * t:128 * (t + 1)]
            pw_np[:, 128 * (2 * b_ + t):128 * (2 * b_ + t + 1)] = blk

    arr1 = w1.T.astype(np.float32)   # [d, df]
    w1_np = np.zeros((128, 2048), dtype=np.float32)
    for c in range(2):
        for t in range(8):
            w1_np[:, 128 * (8 * c + t):128 * (8 * c + t + 1)] = \
                arr1[128 * c:128 * (c + 1), 128 * t:128 * (t + 1)]
    arr2 = w2.T.astype(np.float32)   # [df, d]
    w2_np = np.zeros((128, 2048), dtype=np.float32)
    for c in range(8):
        for t in range(2):
            w2_np[:, 128 * (2 * c + t):128 * (2 * c + t + 1)] = \
                arr2[128 * c:128 * (c + 1), 128 * t:128 * (t + 1)]

    b1_np = b1.reshape(8, 128).T.copy()
    vec2_np = np.zeros((128, 8), dtype=np.float32)
    vec2_np[:, 0:2] = proj_b.reshape(2, 128).T
    vec2_np[:, 2:4] = b2.reshape(2, 128).T
    vec2_np[:, 4:6] = ln1_a.reshape(2, 128).T
    vec2_np[:, 6:8] = ln1_b.reshape(2, 128).T
    ln2_np = np.stack([ln2_a, ln2_b])

    q_const = bool((mask == mask[:, :1, :]).all())
    # k-tiles fully masked for every batch and every query row
    skips = tuple(k for k in range(NKT)
                  if mask[:, :, 128 * k:128 * (k + 1)].all())
    if len(skips) == NKT:
        skips = tuple(range(1, NKT))  # keep one tile so the program is valid
    ln1_trivial = bool((ln1_a == 1).all() and (ln1_b == 0).all())
    ln2_trivial = bool((ln2_a == 1).all() and (ln2_b == 0).all())
    mask_mode = "bias" if q_const else "full"

    cst_np = np.zeros((128, 320), dtype=np.float32)
    cst_np[:, 0:1] = 1.0
    cst_np[:, 1:33] = np.uint32(0x3F803F80).view(np.float32)  # bf16 [1.0,1.0]
    shared = dict(wq=wq_np, wk=wk_np, wv=wv_np, pw=pw_np, w1t=w1_np,
                  w2t=w2_np, b1v=b1_np, vec2=vec2_np, cst=cst_np)
    per_core = []
    for b_ in range(B):
        m = dict(shared)
        m["xT"] = np.ascontiguousarray(x[b_].T)
        if mask_mode == "bias":
            mbv = np.where(mask[b_, 0], MASK_BIAS, 0.0).astype(np.float32)
            m["mb"] = mbv.reshape(8, 128).T.copy()
        else:
            m["mbT"] = np.ascontiguousarray(
                np.where(mask[b_], MASK_BIAS, 0.0).astype(np.float32).T)
        if not ln2_trivial:
            m["ln2v"] = ln2_np
        per_core.append(m)
    flags = (mask_mode, skips, ln1_trivial, ln2_trivial)
    return per_core, flags


def get_program(flags):
    if flags not in _PROGRAM_CACHE:
        _PROGRAM_CACHE[flags] = build_program(*flags)
    return _PROGRAM_CACHE[flags]


def kernel(**inputs):
    per_core, flags = prep_host(inputs)
    nc = get_program(flags)
    res = run_bass_kernel_spmd(nc, per_core, list(range(N_CORES)))
    out = np.stack([res.results[i]["out"] for i in range(N_CORES)])
    return out.astype(np.float32)

